# revision 2
# baseline (speedup 1.0000x reference)
"""GCN (2-layer + MLP head) on 8 NeuronCores — v2: matmul aggregation.

Design (per core, nodes dst-sharded 12500/core, padded to 12800):
  GEMM: hw = (x @ W1) * dinv  (node-major, SBUF-resident self table)
  Feature-major slice [64, 12800] -> AllGather -> table [512, 12800] f32.
  4 chunk phases; chunk k = cores {2k,2k+1} = table rows [128k,128k+128)
  loaded as one SBUF tile [128, 12800] (partitions = 2 cores x 64 feats).
  Edges grouped by (chunk, src-core-half h, dst-block), dst-sorted, each
  (chunk,h,block) run padded to 128 tokens. gpsimd.ap_gather pulls 64
  features of stream-A tokens (partitions 0:64) and stream-B tokens
  (partitions 64:128) per call -> PE transpose -> [128 tok, 128 feats]
  -> M-matmul (M[t,d] = dstrel[t]==d, built on DVE) accumulates into
  PSUM per dst-block run -> folded into SBUF acc.
  Pointwise: h = relu((acc + hw_self)*dinv + b); layer2 GEMM via PE
  transpose; head MLP per tile; output [2, 12800] per core.
"""
import numpy as np

import concourse.bacc as bacc
import concourse.mybir as mybir
from concourse.tile import TileContext
from concourse.bass_utils import run_bass_kernel_spmd
from concourse.masks import make_identity

N = 100000
NS_RAW = 12500
NS = 12800
NTILE = NS // 128          # 100
NBLK = 98                  # blocks with real dsts (12500/128 -> 97.6)
IN_CH, HID, HID2, OUT = 256, 64, 32, 2
TGATH = 2048               # idx per ap_gather call (per stream)
PAD_SENT = -1000.0

_compiled = {}


def _build_schedule(src, dst):
    """Per-core token schedule, shape-equalized across cores.

    The tile/block structure is baked into the compiled SPMD program, so
    every (chunk, stream, block) cell is padded to the max tile count over
    the 8 cores; the per-core data (gather idx, dstrel) differ, the meta
    does not.

    Returns (per_core_data, metas):
      per_core_data[c] = (idx16 [128, TOTI//16] int16,  dstrel [128, NST] f32)
      metas = list per chunk: (LT, tiles_meta); tiles_meta[j] =
              (entA, entB), ent = None | (block, start, stop, first_fold)
    """
    core = dst // NS_RAW
    dstl = (dst % NS_RAW).astype(np.int64)
    src_core = src // NS_RAW
    srcl = (src % NS_RAW).astype(np.int64)
    chunk = src_core // 2
    half = src_core % 2
    blk = dstl // 128

    # per-core sorted cell data + tile counts
    cells = {}          # (c, k, h) -> (bounds, ii, dd)
    ntiles = np.zeros((8, 4, 2, NBLK), np.int64)
    for c in range(8):
        m = core == c
        ch_c, h_c, idx_c, dl_c, b_c = chunk[m], half[m], srcl[m], dstl[m], blk[m]
        for k in range(4):
            for h in range(2):
                mm = (ch_c == k) & (h_c == h)
                o = np.argsort(dl_c[mm], kind="stable")
                ii, dd, bb = idx_c[mm][o], dl_c[mm][o], b_c[mm][o]
                bounds = np.searchsorted(bb, np.arange(NBLK + 1))
                cells[(c, k, h)] = (bounds, ii, dd)
                ntiles[c, k, h] = np.ceil(
                    (bounds[1:] - bounds[:-1]) / 128).astype(np.int64)

    # common tile count per (k, h, block): max over cores (>=1 if any core
    # has tokens there; 0 only if NO core does)
    ct = ntiles.max(axis=0)           # [4, 2, NBLK]
    LTs = [int(max(ct[k, 0].sum(), ct[k, 1].sum(), 1)) for k in range(4)]

    # meta (core-independent)
    metas = []
    folded_first = set()
    for k in range(4):
        LT = LTs[k]
        tbs = []
        for h in range(2):
            tb = []
            for b in range(NBLK):
                tb.extend([b] * int(ct[k, h, b]))
            tb += [-1] * (LT - len(tb))
            tbs.append(tb)
        tiles_meta = []
        for j in range(LT):
            ent = []
            for h in range(2):
                tb = tbs[h]
                b = tb[j]
                if b < 0:
                    ent.append(None)
                    continue
                start = j == 0 or tb[j - 1] != b
                stop = j == LT - 1 or tb[j + 1] != b
                first = False
                if stop:
                    key = (h * NBLK + b) if False else b
                    first = key not in folded_first
                    folded_first.add(key)
                ent.append((b, start, stop, first))
            tiles_meta.append(tuple(ent))
        metas.append((LT, tiles_meta))

    # per-core arrays following the common shape
    per_core = []
    for c in range(8):
        idx_cols, dr_cols = [], []
        for k in range(4):
            LT = LTs[k]
            L = LT * 128
            ihs, dhs = [], []
            for h in range(2):
                bounds, ii, dd = cells[(c, k, h)]
                i_parts, d_parts = [], []
                used = 0
                for b in range(NBLK):
                    nt = int(ct[k, h, b])
                    if nt == 0:
                        continue
                    lo, hi = bounds[b], bounds[b + 1]
                    n = hi - lo
                    cap = nt * 128
                    i_parts.append(ii[lo:hi])
                    d_parts.append(dd[lo:hi] - 128 * b)
                    if cap > n:
                        i_parts.append(np.zeros(cap - n, np.int64))
                        d_parts.append(np.full(cap - n, PAD_SENT))
                    used += cap
                if L > used:
                    i_parts.append(np.zeros(L - used, np.int64))
                    d_parts.append(np.full(L - used, PAD_SENT))
                ia = np.concatenate(i_parts) if i_parts else np.zeros(L, np.int64)
                da = np.concatenate(d_parts) if d_parts else np.full(L, PAD_SENT)
                ihs.append(ia)
                dhs.append(da)
            wA = ihs[0].reshape(-1, 16).T.astype(np.int16)
            wB = ihs[1].reshape(-1, 16).T.astype(np.int16)
            idx_cols.append(np.concatenate([np.tile(wA, (4, 1)),
                                            np.tile(wB, (4, 1))], axis=0))
            drk = np.empty((128, 2 * LT), np.float32)
            drk[:, 0::2] = dhs[0].reshape(LT, 128).T
            drk[:, 1::2] = dhs[1].reshape(LT, 128).T
            dr_cols.append(drk)
        per_core.append((np.concatenate(idx_cols, axis=1),
                         np.concatenate(dr_cols, axis=1)))
    return per_core, metas


def _build_program(metas):
    """metas: per-chunk (LT, tiles_meta) — identical structure for both layers."""
    nc = bacc.Bacc(None, target_bir_lowering=False)
    dt = mybir.dt
    P = nc.declare_dram_parameter
    NST = sum(2 * LT for LT, _ in metas)
    TOTI = sum(LT * 128 for LT, _ in metas)
    xT = P("xT", [IN_CH, NS], dt.float32, isOutput=False)
    w1p = P("w1p", [128, 128], dt.float32, isOutput=False)
    w2 = P("w2", [HID, HID], dt.float32, isOutput=False)
    wh1 = P("wh1", [HID, HID2], dt.float32, isOutput=False)
    wh2 = P("wh2", [HID2, OUT], dt.float32, isOutput=False)
    b1f = P("b1f", [128, HID], dt.float32, isOutput=False)
    b2f = P("b2f", [128, HID], dt.float32, isOutput=False)
    bh1 = P("bh1", [HID2, 1], dt.float32, isOutput=False)
    bh2 = P("bh2", [OUT, 1], dt.float32, isOutput=False)
    dinvP = P("dinvP", [128, NTILE], dt.float32, isOutput=False)
    idxP = P("idxP", [128, TOTI // 16], dt.int16, isOutput=False)
    dstrelP = P("dstrelP", [128, NST], dt.float32, isOutput=False)
    iotaP = P("iotaP", [128, 1024], dt.float32, isOutput=False)
    outT = P("outT", [OUT, NS], dt.float32, isOutput=True)

    sliceT = [nc.dram_tensor(f"sliceT{l}", [HID, NS], dt.float32) for l in (1, 2)]
    tableT = [nc.dram_tensor(f"tableT{l}", [8 * HID, NS], dt.float32) for l in (1, 2)]

    add = mybir.AluOpType.add
    iseq = mybir.AluOpType.is_equal
    relu = mybir.ActivationFunctionType.Relu
    copyf = mybir.ActivationFunctionType.Copy

    with TileContext(nc) as tc:
        with tc.tile_pool(name="const", bufs=1) as cp, \
             tc.tile_pool(name="tab", bufs=1) as tbp, \
             tc.tile_pool(name="acc", bufs=1) as ap_, \
             tc.tile_pool(name="gath", bufs=3) as gp, \
             tc.tile_pool(name="work", bufs=3) as wp, \
             tc.tile_pool(name="m8", bufs=3) as mp, \
             tc.tile_pool(name="idx", bufs=3) as ip, \
             tc.tile_pool(name="pst", bufs=2, space="PSUM") as ptp, \
             tc.tile_pool(name="psa", bufs=2, space="PSUM") as pap, \
             tc.tile_pool(name="psg", bufs=2, space="PSUM") as pgp:
            # ---- constants ----
            w1sb = cp.tile([128, 128], dt.float32)
            nc.sync.dma_start(out=w1sb[:], in_=w1p[:])
            w2sb = cp.tile([HID, HID], dt.float32)
            nc.sync.dma_start(out=w2sb[:], in_=w2[:])
            wh1sb = cp.tile([HID, HID2], dt.float32)
            nc.sync.dma_start(out=wh1sb[:], in_=wh1[:])
            wh2sb = cp.tile([HID2, OUT], dt.float32)
            nc.sync.dma_start(out=wh2sb[:], in_=wh2[:])
            b1sb = cp.tile([128, HID], dt.float32)
            nc.sync.dma_start(out=b1sb[:], in_=b1f[:])
            b2sb = cp.tile([128, HID], dt.float32)
            nc.sync.dma_start(out=b2sb[:], in_=b2f[:])
            bh1sb = cp.tile([HID2, 1], dt.float32)
            nc.sync.dma_start(out=bh1sb[:], in_=bh1[:])
            bh2sb = cp.tile([OUT, 1], dt.float32)
            nc.sync.dma_start(out=bh2sb[:], in_=bh2[:])
            dsb = cp.tile([128, NTILE], dt.float32)
            nc.sync.dma_start(out=dsb[:], in_=dinvP[:])
            ident = cp.tile([128, 128], dt.float32)
            make_identity(nc, ident[:])
            iotab = cp.tile([128, 1024], dt.bfloat16)
            iotaf = cp.tile([128, 1024], dt.float32)
            nc.sync.dma_start(out=iotaf[:], in_=iotaP[:])
            nc.vector.tensor_copy(iotab[:], iotaf[:])
            drb = cp.tile([128, NST], dt.bfloat16)
            drf = cp.tile([128, NST], dt.float32)
            nc.sync.dma_start(out=drf[:], in_=dstrelP[:])
            nc.vector.tensor_copy(drb[:], drf[:])

            tabsb = tbp.tile([128, NS], dt.float32)
            acc = ap_.tile([128, NBLK * HID], dt.float32)
            hwself = ap_.tile([128, NTILE * HID], dt.float32)

            def gemm1():
                for m in range(NTILE):
                    mc = slice(m * 128, (m + 1) * 128)
                    xa = wp.tile([128, 128], dt.float32, tag="xa")
                    nc.sync.dma_start(out=xa[:], in_=xT[0:128, mc])
                    xb = wp.tile([128, 128], dt.float32, tag="xb")
                    nc.sync.dma_start(out=xb[:], in_=xT[128:256, mc])
                    ps = pgp.tile([128, HID], dt.float32, tag="ps")
                    nc.tensor.matmul(ps[:], xa[:], w1sb[:, 0:HID],
                                     start=True, stop=False)
                    nc.tensor.matmul(ps[:], xb[:], w1sb[:, HID:128],
                                     start=False, stop=True)
                    hsl = hwself[:, m * HID:(m + 1) * HID]
                    nc.vector.tensor_scalar_mul(hsl, ps[:], dsb[:, m:m + 1])
                    pt = ptp.tile([128, 128], dt.float32, tag="tp")
                    nc.tensor.transpose(pt[0:HID, :], hsl, ident[:])
                    st = wp.tile([HID, 128], dt.float32, tag="st")
                    nc.scalar.activation(st[:], pt[0:HID, :], copyf)
                    nc.sync.dma_start(out=sliceT[0][:, mc], in_=st[:])

            def aggregate(layer):
                tbl = tableT[layer]
                goff = 0      # stream-tile counter (dstrel column)
                ioff = 0      # idx offset (int16 elements per partition row)
                for k in range(4):
                    LT, tiles_meta = metas[k]
                    nc.sync.dma_start(out=tabsb[:],
                                      in_=tbl[128 * k:128 * (k + 1), :])
                    cur = [None, None]    # live psum agg tile per stream
                    j = 0
                    while j < LT:
                        T = min(TGATH // 128, LT - j) * 128
                        idxt = ip.tile([128, TGATH // 16], dt.int16, tag="ix")
                        nc.sync.dma_start(
                            out=idxt[:, 0:T // 16],
                            in_=idxP[:, ioff:ioff + T // 16])
                        g = gp.tile([128, TGATH], dt.float32, tag="g")
                        nc.gpsimd.ap_gather(
                            g[:, 0:T].rearrange("p (n d) -> p n d", d=1),
                            tabsb[:].rearrange("p (n d) -> p n d", d=1),
                            idxt[:, 0:T // 16], 128, NS, 1, T)
                        for jj in range(T // 128):
                            tm = tiles_meta[j + jj]
                            if goff % 8 == 0:
                                m8 = mp.tile([128, 1024], dt.bfloat16, tag="m8")
                                nw = min(8, NST - goff)
                                d3 = drb[:, goff:goff + nw]
                                d3b = d3.rearrange("p (k o) -> p k o", o=1) \
                                        .broadcast_to((128, nw, 128))
                                nc.vector.tensor_tensor(
                                    m8[:, 0:nw * 128].rearrange(
                                        "p (k o) -> p k o", o=128),
                                    iotab[:, 0:nw * 128].rearrange(
                                        "p (k o) -> p k o", o=128),
                                    d3b, iseq)
                            if tm == (None, None):
                                goff += 2
                                continue
                            gsl = g[:, jj * 128:(jj + 1) * 128]
                            ptt = ptp.tile([128, 128], dt.float32, tag="tp")
                            nc.tensor.transpose(ptt[:], gsl, ident[:])
                            gtb = wp.tile([128, 128], dt.bfloat16, tag="gtb")
                            nc.scalar.activation(gtb[:], ptt[:], copyf)
                            for s in range(2):
                                ent = tm[s]
                                gcol = goff % 8
                                goff += 1
                                if ent is None:
                                    continue
                                b, start, stop, first = ent
                                if start:
                                    agg_t = pap.tile([128, HID], dt.float32,
                                                     tag=f"agg{s}")
                                    cur[s] = agg_t
                                nc.tensor.matmul(
                                    cur[s][:],
                                    m8[:, gcol * 128:(gcol + 1) * 128],
                                    gtb[:, s * HID:(s + 1) * HID],
                                    start=start, stop=stop)
                                if stop:
                                    asl = acc[:, b * HID:(b + 1) * HID]
                                    if first:
                                        nc.vector.tensor_copy(asl, cur[s][:])
                                    else:
                                        nc.vector.tensor_add(asl, asl, cur[s][:])
                        ioff += T // 16
                        j += T // 128

            def pointwise(layer):
                """layer 1: h=relu(..b1); hw2=(h@W2)*dinv -> hwself, sliceT2.
                   layer 2: h2=relu(..b2); head -> outT."""
                for m in range(NTILE):
                    mc = slice(m * 128, (m + 1) * 128)
                    hsl = hwself[:, m * HID:(m + 1) * HID]
                    s = wp.tile([128, HID], dt.float32, tag="s")
                    if m * 128 < NBLK * 128:
                        asl = acc[:, m * HID:(m + 1) * HID]
                        nc.vector.tensor_add(s[:], asl, hsl)
                    else:
                        nc.vector.tensor_copy(s[:], hsl)
                    nc.vector.tensor_scalar_mul(s[:], s[:], dsb[:, m:m + 1])
                    nc.vector.tensor_add(s[:], s[:],
                                         b1sb[:] if layer == 0 else b2sb[:])
                    h = wp.tile([128, HID], dt.float32, tag="h")
                    nc.scalar.activation(h[:], s[:], relu)
                    pt = ptp.tile([128, 128], dt.float32, tag="tp")
                    nc.tensor.transpose(pt[0:HID, :], h[:], ident[:])
                    ht = wp.tile([HID, 128], dt.float32, tag="ht")
                    nc.scalar.activation(ht[:], pt[0:HID, :], copyf)
                    if layer == 0:
                        ps2 = pgp.tile([128, HID], dt.float32, tag="ps")
                        nc.tensor.matmul(ps2[:], ht[:], w2sb[:],
                                         start=True, stop=True)
                        nc.vector.tensor_scalar_mul(hsl, ps2[:], dsb[:, m:m + 1])
                        pt2 = ptp.tile([128, 128], dt.float32, tag="tp")
                        nc.tensor.transpose(pt2[0:HID, :], hsl, ident[:])
                        st2 = wp.tile([HID, 128], dt.float32, tag="st2")
                        nc.scalar.activation(st2[:], pt2[0:HID, :], copyf)
                        nc.sync.dma_start(out=sliceT[1][:, mc], in_=st2[:])
                    else:
                        pz = ptp.tile([128, 128], dt.float32, tag="tp")
                        nc.tensor.matmul(pz[0:HID2, :], wh1sb[:], ht[:],
                                         start=True, stop=True)
                        zb = wp.tile([HID2, 128], dt.float32, tag="zb")
                        nc.scalar.activation(zb[:], pz[0:HID2, :], relu,
                                             bias=bh1sb[:])
                        po = ptp.tile([128, 128], dt.float32, tag="tp")
                        nc.tensor.matmul(po[0:OUT, :], wh2sb[:], zb[:],
                                         start=True, stop=True)
                        ob = wp.tile([OUT, 128], dt.float32, tag="ob")
                        nc.vector.tensor_scalar_add(ob[:], po[0:OUT, :],
                                                    bh2sb[:])
                        nc.sync.dma_start(out=outT[:, mc], in_=ob[:])

            gemm1()
            nc.gpsimd.collective_compute(
                "AllGather", mybir.AluOpType.bypass,
                replica_groups=[list(range(8))],
                ins=[sliceT[0][:]], outs=[tableT[0][:]])
            aggregate(0)
            pointwise(0)
            nc.gpsimd.collective_compute(
                "AllGather", mybir.AluOpType.bypass,
                replica_groups=[list(range(8))],
                ins=[sliceT[1][:]], outs=[tableT[1][:]])
            aggregate(1)
            pointwise(1)

    nc.finalize()
    return nc


def kernel(x, edge_index, W1, b1, W2, b2, Wh1, bh1, Wh2, bh2, _trace=False):
    x = np.asarray(x, np.float32)
    src = np.asarray(edge_index[0], np.int64)
    dst = np.asarray(edge_index[1], np.int64)

    per_core, metas = _build_schedule(src, dst)
    sig = tuple((LT, tuple(tm)) for LT, tm in metas)
    if sig not in _compiled:
        _compiled[sig] = _build_program(metas)
    nc = _compiled[sig]

    deg = np.bincount(dst, minlength=N).astype(np.float64) + 1.0
    dinv = (1.0 / np.sqrt(deg)).astype(np.float32)

    W1 = np.asarray(W1, np.float32)
    w1p = np.concatenate([W1[:128], W1[128:]], axis=1)
    b1f = np.tile(np.asarray(b1, np.float32)[None, :], (128, 1))
    b2f = np.tile(np.asarray(b2, np.float32)[None, :], (128, 1))
    bh1c = np.asarray(bh1, np.float32)[:, None]
    bh2c = np.asarray(bh2, np.float32)[:, None]
    iota = np.tile(np.arange(128, dtype=np.float32)[None, :], (128, 8))

    in_maps = []
    for c in range(8):
        idx16, dstrel = per_core[c]
        xs = np.zeros((NS, IN_CH), np.float32)
        xs[:NS_RAW] = x[c * NS_RAW:(c + 1) * NS_RAW]
        dv = np.ones(NS, np.float32)
        dv[:NS_RAW] = dinv[c * NS_RAW:(c + 1) * NS_RAW]
        in_maps.append({
            "xT": np.ascontiguousarray(xs.T),
            "w1p": np.ascontiguousarray(w1p),
            "w2": np.asarray(W2, np.float32),
            "wh1": np.asarray(Wh1, np.float32),
            "wh2": np.asarray(Wh2, np.float32),
            "b1f": b1f, "b2f": b2f, "bh1": bh1c, "bh2": bh2c,
            "dinvP": np.ascontiguousarray(dv.reshape(NTILE, 128).T),
            "idxP": np.ascontiguousarray(idx16),
            "dstrelP": np.ascontiguousarray(dstrel),
            "iotaP": iota,
        })

    res = run_bass_kernel_spmd(nc, in_maps, list(range(8)), trace=_trace)
    out = np.empty((N, OUT), np.float32)
    for c in range(8):
        out[c * NS_RAW:(c + 1) * NS_RAW] = res.results[c]["outT"].T[:NS_RAW]
    if _trace:
        kernel.last_results = res
    return out


# revision 3
# speedup vs baseline: 1.0536x; 1.0536x over previous
"""GCN (2-layer + MLP head) on 8 NeuronCores — v3: indirect-DMA gather +
matmul aggregation.

Per core (nodes dst-sharded, 12500 real / 12800 padded):
  GEMM: hw = (x @ W1) * dinv  (node-major, fp32, SBUF-resident self table)
  slice [12800, 64] bf16 -> AllGather -> table [102400, 64] bf16 in DRAM.
  Edge tokens dst-sorted, grouped by dst-block (128 dsts), each block's
  token count padded to a cross-core-common multiple of 128.
  gpsimd.indirect_dma_start gathers 8192 tokens/call from the full table
  (HW DGE, int32 row indices, token-major out [128, 64, 64] bf16).
  Aggregation: per token-tile, M[t,d] = (dstrel[t]==d) built on DVE
  (is_equal vs iota, bf16), PE matmul M^T @ g accumulates a dst-block's
  tiles in PSUM; one fold per block into SBUF acc (fp32).
  Pointwise: h = relu((acc + hw_self)*dinv + b); layer-2 GEMM via PE
  transpose of h; head MLP per tile; out [2, 12800] per core.
"""
import numpy as np

import concourse.bacc as bacc
import concourse.mybir as mybir
from concourse import bass
from concourse.tile import TileContext
from concourse.bass_utils import run_bass_kernel_spmd
from concourse.masks import make_identity

N = 100000
NS_RAW = 12500
NS = 12800
NTILE = NS // 128          # 100
NBLK = 98                  # blocks containing real dsts
IN_CH, HID, HID2, OUT = 256, 64, 32, 2
NT_CALL = 64               # gather-call size in token-tiles (8192 tokens)
PAD_SENT = -1000.0

_compiled = {}


def _build_schedule(src, dst):
    """Token schedule, shape-equalized across cores.

    Returns (per_core, tbs):
      per_core[c] = (idx32 [128, TOTC] int32 table-row indices,
                     dstrel [128, TOTC] f32 dst-local-in-block / PAD_SENT)
      tbs = [tiles per block] (common across cores), sum = TOTC
    """
    core = dst // NS_RAW
    dstl = (dst % NS_RAW).astype(np.int64)
    rows = (src // NS_RAW) * NS + (src % NS_RAW)   # padded table row
    blk = dstl // 128

    cells = {}
    ntile = np.zeros((8, NBLK), np.int64)
    for c in range(8):
        m = core == c
        o = np.argsort(dstl[m], kind="stable")
        rr, dd, bb = rows[m][o], dstl[m][o], blk[m][o]
        bounds = np.searchsorted(bb, np.arange(NBLK + 1))
        cells[c] = (bounds, rr, dd)
        ntile[c] = np.ceil((bounds[1:] - bounds[:-1]) / 128).astype(np.int64)

    tbs = ntile.max(axis=0)            # tiles per block, common
    assert tbs.min() >= 1
    TOTC = int(tbs.sum())

    per_core = []
    for c in range(8):
        bounds, rr, dd = cells[c]
        r_parts, d_parts = [], []
        for b in range(NBLK):
            lo, hi = bounds[b], bounds[b + 1]
            n = hi - lo
            cap = int(tbs[b]) * 128
            r_parts.append(rr[lo:hi])
            d_parts.append(dd[lo:hi] - 128 * b)
            if cap > n:
                r_parts.append(np.zeros(cap - n, np.int64))
                d_parts.append(np.full(cap - n, PAD_SENT))
        ra = np.concatenate(r_parts)
        da = np.concatenate(d_parts)
        # token s -> (partition s%128, column s//128)
        idx32 = ra.reshape(TOTC, 128).T.astype(np.int32)
        drel = da.reshape(TOTC, 128).T.astype(np.float32)
        per_core.append((np.ascontiguousarray(idx32),
                         np.ascontiguousarray(drel)))
    return per_core, [int(t) for t in tbs]


def _build_program(tbs):
    nc = bacc.Bacc(None, target_bir_lowering=False)
    dt = mybir.dt
    P = nc.declare_dram_parameter
    TOTC = sum(tbs)
    xT = P("xT", [IN_CH, NS], dt.float32, isOutput=False)
    w1p = P("w1p", [128, 128], dt.float32, isOutput=False)
    w2 = P("w2", [HID, HID], dt.float32, isOutput=False)
    wh1 = P("wh1", [HID, HID2], dt.float32, isOutput=False)
    wh2 = P("wh2", [HID2, OUT], dt.float32, isOutput=False)
    b1f = P("b1f", [128, HID], dt.float32, isOutput=False)
    b2f = P("b2f", [128, HID], dt.float32, isOutput=False)
    bh1 = P("bh1", [HID2, 1], dt.float32, isOutput=False)
    bh2 = P("bh2", [OUT, 1], dt.float32, isOutput=False)
    dinvP = P("dinvP", [128, NTILE], dt.float32, isOutput=False)
    idxP = P("idxP", [128, TOTC], dt.int32, isOutput=False)
    dstrelP = P("dstrelP", [128, TOTC], dt.float32, isOutput=False)
    iotaP = P("iotaP", [128, 1024], dt.float32, isOutput=False)
    outT = P("outT", [OUT, NS], dt.float32, isOutput=True)

    slice_d = [nc.dram_tensor(f"slice{l}", [NS, HID], dt.bfloat16) for l in (1, 2)]
    table_d = [nc.dram_tensor(f"table{l}", [8 * NS, HID], dt.bfloat16)
               for l in (1, 2)]

    iseq = mybir.AluOpType.is_equal
    relu = mybir.ActivationFunctionType.Relu
    copyf = mybir.ActivationFunctionType.Copy

    # block -> column range
    boff = np.cumsum([0] + list(tbs))

    with TileContext(nc) as tc:
        with tc.tile_pool(name="const", bufs=1) as cp, \
             tc.tile_pool(name="acc", bufs=1) as ap_, \
             tc.tile_pool(name="gath", bufs=3) as gp, \
             tc.tile_pool(name="work", bufs=3) as wp, \
             tc.tile_pool(name="m8", bufs=3) as mp, \
             tc.tile_pool(name="pst", bufs=2, space="PSUM") as ptp, \
             tc.tile_pool(name="psa", bufs=2, space="PSUM") as pap, \
             tc.tile_pool(name="psg", bufs=2, space="PSUM") as pgp:
            w1sb = cp.tile([128, 128], dt.float32)
            nc.sync.dma_start(out=w1sb[:], in_=w1p[:])
            w2sb = cp.tile([HID, HID], dt.float32)
            nc.sync.dma_start(out=w2sb[:], in_=w2[:])
            wh1sb = cp.tile([HID, HID2], dt.float32)
            nc.sync.dma_start(out=wh1sb[:], in_=wh1[:])
            wh2sb = cp.tile([HID2, OUT], dt.float32)
            nc.sync.dma_start(out=wh2sb[:], in_=wh2[:])
            b1sb = cp.tile([128, HID], dt.float32)
            nc.sync.dma_start(out=b1sb[:], in_=b1f[:])
            b2sb = cp.tile([128, HID], dt.float32)
            nc.sync.dma_start(out=b2sb[:], in_=b2f[:])
            bh1sb = cp.tile([HID2, 1], dt.float32)
            nc.sync.dma_start(out=bh1sb[:], in_=bh1[:])
            bh2sb = cp.tile([OUT, 1], dt.float32)
            nc.sync.dma_start(out=bh2sb[:], in_=bh2[:])
            dsb = cp.tile([128, NTILE], dt.float32)
            nc.sync.dma_start(out=dsb[:], in_=dinvP[:])
            ident = cp.tile([128, 128], dt.float32)
            make_identity(nc, ident[:])
            iotab = cp.tile([128, 1024], dt.bfloat16)
            iotaf = cp.tile([128, 1024], dt.float32)
            nc.sync.dma_start(out=iotaf[:], in_=iotaP[:])
            nc.vector.tensor_copy(iotab[:], iotaf[:])
            drb = cp.tile([128, TOTC], dt.bfloat16)
            drf = cp.tile([128, TOTC], dt.float32)
            nc.sync.dma_start(out=drf[:], in_=dstrelP[:])
            nc.vector.tensor_copy(drb[:], drf[:])
            idxsb = cp.tile([128, TOTC], dt.int32)
            nc.sync.dma_start(out=idxsb[:], in_=idxP[:])

            acc = ap_.tile([128, NBLK * HID], dt.float32)
            hwself = ap_.tile([128, NTILE * HID], dt.float32)

            def gemm1():
                for m in range(NTILE):
                    mc = slice(m * 128, (m + 1) * 128)
                    xa = wp.tile([128, 128], dt.float32, tag="xa")
                    nc.sync.dma_start(out=xa[:], in_=xT[0:128, mc])
                    xb = wp.tile([128, 128], dt.float32, tag="xb")
                    nc.sync.dma_start(out=xb[:], in_=xT[128:256, mc])
                    ps = pgp.tile([128, HID], dt.float32, tag="ps")
                    nc.tensor.matmul(ps[:], xa[:], w1sb[:, 0:HID],
                                     start=True, stop=False)
                    nc.tensor.matmul(ps[:], xb[:], w1sb[:, HID:128],
                                     start=False, stop=True)
                    hsl = hwself[:, m * HID:(m + 1) * HID]
                    nc.vector.tensor_scalar_mul(hsl, ps[:], dsb[:, m:m + 1])
                    sb16 = wp.tile([128, HID], dt.bfloat16, tag="sb16")
                    nc.scalar.activation(sb16[:], hsl, copyf)
                    nc.sync.dma_start(out=slice_d[0][mc, :], in_=sb16[:])

            def aggregate(layer):
                tbl = table_d[layer]
                # block schedule per column
                col_block = []
                for b in range(NBLK):
                    col_block += [b] * tbs[b]
                for c in range(TOTC):
                    g = gp.tile([128, HID], dt.bfloat16, tag="g")
                    nc.gpsimd.indirect_dma_start(
                        out=g[:],
                        out_offset=None,
                        in_=tbl[:],
                        in_offset=bass.IndirectOffsetOnAxis(
                            ap=idxsb[:, c:c + 1], axis=0),
                    )
                    if c % 8 == 0:
                        m8 = mp.tile([128, 1024], dt.bfloat16, tag="m8")
                        nw = min(8, TOTC - c)
                        d3b = drb[:, c:c + nw] \
                            .rearrange("p (k o) -> p k o", o=1) \
                            .broadcast_to((128, nw, 128))
                        nc.vector.tensor_tensor(
                            m8[:, 0:nw * 128].rearrange(
                                "p (k o) -> p k o", o=128),
                            iotab[:, 0:nw * 128].rearrange(
                                "p (k o) -> p k o", o=128),
                            d3b, iseq)
                    b = col_block[c]
                    start = c == boff[b]
                    stop = c == boff[b + 1] - 1
                    if start:
                        agg_t = pap.tile([128, HID], dt.float32, tag="agg")
                        cur = agg_t
                    nc.tensor.matmul(
                        cur[:],
                        m8[:, (c % 8) * 128:(c % 8 + 1) * 128],
                        g[:],
                        start=start, stop=stop)
                    if stop:
                        nc.vector.tensor_copy(
                            acc[:, b * HID:(b + 1) * HID], cur[:])

            def pointwise(layer):
                for m in range(NTILE):
                    mc = slice(m * 128, (m + 1) * 128)
                    hsl = hwself[:, m * HID:(m + 1) * HID]
                    s = wp.tile([128, HID], dt.float32, tag="s")
                    if m < NBLK:
                        nc.vector.tensor_add(
                            s[:], acc[:, m * HID:(m + 1) * HID], hsl)
                    else:
                        nc.vector.tensor_copy(s[:], hsl)
                    nc.vector.tensor_scalar_mul(s[:], s[:], dsb[:, m:m + 1])
                    nc.vector.tensor_add(s[:], s[:],
                                         b1sb[:] if layer == 0 else b2sb[:])
                    h = wp.tile([128, HID], dt.float32, tag="h")
                    nc.scalar.activation(h[:], s[:], relu)
                    pt = ptp.tile([128, 128], dt.float32, tag="tp")
                    nc.tensor.transpose(pt[0:HID, :], h[:], ident[:])
                    ht = wp.tile([HID, 128], dt.float32, tag="ht")
                    nc.scalar.activation(ht[:], pt[0:HID, :], copyf)
                    if layer == 0:
                        ps2 = pgp.tile([128, HID], dt.float32, tag="ps")
                        nc.tensor.matmul(ps2[:], ht[:], w2sb[:],
                                         start=True, stop=True)
                        nc.vector.tensor_scalar_mul(hsl, ps2[:],
                                                    dsb[:, m:m + 1])
                        sb16 = wp.tile([128, HID], dt.bfloat16, tag="sb16b")
                        nc.scalar.activation(sb16[:], hsl, copyf)
                        nc.sync.dma_start(out=slice_d[1][mc, :], in_=sb16[:])
                    else:
                        pz = ptp.tile([128, 128], dt.float32, tag="tp")
                        nc.tensor.matmul(pz[0:HID2, :], wh1sb[:], ht[:],
                                         start=True, stop=True)
                        zb = wp.tile([HID2, 128], dt.float32, tag="zb")
                        nc.scalar.activation(zb[:], pz[0:HID2, :], relu,
                                             bias=bh1sb[:])
                        po = ptp.tile([128, 128], dt.float32, tag="tp")
                        nc.tensor.matmul(po[0:OUT, :], wh2sb[:], zb[:],
                                         start=True, stop=True)
                        ob = wp.tile([OUT, 128], dt.float32, tag="ob")
                        nc.vector.tensor_scalar_add(ob[:], po[0:OUT, :],
                                                    bh2sb[:])
                        nc.sync.dma_start(out=outT[:, mc], in_=ob[:])

            gemm1()
            nc.gpsimd.collective_compute(
                "AllGather", mybir.AluOpType.bypass,
                replica_groups=[list(range(8))],
                ins=[slice_d[0][:]], outs=[table_d[0][:]])
            aggregate(0)
            pointwise(0)
            nc.gpsimd.collective_compute(
                "AllGather", mybir.AluOpType.bypass,
                replica_groups=[list(range(8))],
                ins=[slice_d[1][:]], outs=[table_d[1][:]])
            aggregate(1)
            pointwise(1)

    nc.finalize()
    return nc


def kernel(x, edge_index, W1, b1, W2, b2, Wh1, bh1, Wh2, bh2, _trace=False):
    x = np.asarray(x, np.float32)
    src = np.asarray(edge_index[0], np.int64)
    dst = np.asarray(edge_index[1], np.int64)

    per_core, tbs = _build_schedule(src, dst)
    sig = tuple(tbs)
    if sig not in _compiled:
        _compiled[sig] = _build_program(tbs)
    nc = _compiled[sig]

    deg = np.bincount(dst, minlength=N).astype(np.float64) + 1.0
    dinv = (1.0 / np.sqrt(deg)).astype(np.float32)

    W1 = np.asarray(W1, np.float32)
    w1p = np.concatenate([W1[:128], W1[128:]], axis=1)
    b1f = np.tile(np.asarray(b1, np.float32)[None, :], (128, 1))
    b2f = np.tile(np.asarray(b2, np.float32)[None, :], (128, 1))
    bh1c = np.asarray(bh1, np.float32)[:, None]
    bh2c = np.asarray(bh2, np.float32)[:, None]
    iota = np.tile(np.arange(128, dtype=np.float32)[None, :], (128, 8))

    in_maps = []
    for c in range(8):
        idx32, dstrel = per_core[c]
        xs = np.zeros((NS, IN_CH), np.float32)
        xs[:NS_RAW] = x[c * NS_RAW:(c + 1) * NS_RAW]
        dv = np.ones(NS, np.float32)
        dv[:NS_RAW] = dinv[c * NS_RAW:(c + 1) * NS_RAW]
        in_maps.append({
            "xT": np.ascontiguousarray(xs.T),
            "w1p": np.ascontiguousarray(w1p),
            "w2": np.asarray(W2, np.float32),
            "wh1": np.asarray(Wh1, np.float32),
            "wh2": np.asarray(Wh2, np.float32),
            "b1f": b1f, "b2f": b2f, "bh1": bh1c, "bh2": bh2c,
            "dinvP": np.ascontiguousarray(dv.reshape(NTILE, 128).T),
            "idxP": idx32,
            "dstrelP": dstrel,
            "iotaP": iota,
        })

    res = run_bass_kernel_spmd(nc, in_maps, list(range(8)), trace=_trace)
    out = np.empty((N, OUT), np.float32)
    for c in range(8):
        out[c * NS_RAW:(c + 1) * NS_RAW] = res.results[c]["outT"].T[:NS_RAW]
    if _trace:
        kernel.last_results = res
    return out
